# revision 64
# baseline (speedup 1.0000x reference)
"""MSRSA multi-head attention kernel for 8 Trainium2 NeuronCores.

Strategy: data-parallel over batch (B=8 -> 1 batch element per core).
Per core, for its batch element b:
  Qt = (W_q/8) @ queries^T        [512,1024]  (scale 1/8 folded into W_q)
  Kt = W_k @ keys^T               [512,1024]
  V  = values @ W_v^T             [1024,512]  (rows masked by attention_mask)
  per head h, scores are computed TRANSPOSED: S_T[k,q]:
     S_T = sum_d Kt[d,k]*Qt[d,q] + B_h^T[k,q]
  where B_h = la[h]*A + ld[h]*D is combined HOST-side per head (fp16) and
  injected into PSUM by a single identity matmul per k-tile (vs 2 scaled-
  identity matmuls in v1 -- halves bias PE cycles, 16MB/core DMA streamed
  per-head double-buffered).
  expS = exp(S_T) on ScalarE (PSUM -> SBUF evacuation is the exp)
  attnT_h[d,q] (+ denominator row) = sum_k V_ext[k, d|mask] * expS[k,q]
  (mask column of V_ext -> row 64 of PV output = softmax denominator)
  normalize via reciprocal_approx_fast (DVE) + K=1 ones-matmul broadcast;
  the broadcast matmul is EMITTED after the next head's QK block so the PE
  never idles waiting on the DVE reciprocal (also keeps the PE p-state at
  full clock).
  out = attnT contracted with W_o^T   [1024, 512]

Matmul operands are fp16 (1 PE cycle/row); accumulation fp32 in PSUM.
exp and softmax normalization run in fp32. Transposes host-side.
"""

import contextlib

import numpy as np

import concourse.bass as bass
import concourse.mybir as mybir
import concourse.tile as tile
from concourse.bass_utils import run_bass_kernel_spmd

B, L, DIN, DM, H = 8, 1024, 256, 512, 8
DH = DM // H  # 64
P = 128
NKT = L // P          # 8 k-tiles
NQC = 2               # q chunks
QC = L // NQC         # 512
F32 = mybir.dt.float32
F16 = mybir.dt.float16


def _emit(tc):
    nc = tc.nc

    def dram(name, shape, dtype=F16, kind="ExternalInput"):
        return nc.dram_tensor(name, shape, dtype, kind=kind).ap()

    qT = dram("qT", [DIN, L])
    kT = dram("kT", [DIN, L])
    vT = dram("vT", [DIN, L])
    wqT = dram("wqT", [DIN, DM])
    wkT = dram("wkT", [DIN, DM])
    wvT = dram("wvT", [DIN, DM])
    woT = dram("woT", [DM, DM])
    bT = dram("bT", [H * L, L])   # per-head combined bias, transposed
    identI = dram("identI", [P, P])
    mask01 = dram("mask01", [P, NKT], F32)
    out = dram("out", [L, DM], F32, kind="ExternalOutput")

    bT_r = bT.rearrange("(h t p) q -> p h t q", p=P, t=NKT)

    with contextlib.ExitStack() as ctx:
        singles = ctx.enter_context(tc.tile_pool(name="singles", bufs=1))
        big = ctx.enter_context(tc.tile_pool(name="big", bufs=1))
        bpool = ctx.enter_context(tc.tile_pool(name="bpool", bufs=3))
        exps = ctx.enter_context(tc.tile_pool(name="exps", bufs=3))
        small = ctx.enter_context(tc.tile_pool(name="small", bufs=2))
        spsum = ctx.enter_context(tc.tile_pool(name="spsum", bufs=2, space="PSUM"))
        pvp = ctx.enter_context(tc.tile_pool(name="pvp", bufs=2, space="PSUM"))
        bcp = ctx.enter_context(tc.tile_pool(name="bcp", bufs=2, space="PSUM"))

        # ---- small constants (DMAs issued late; they gate nothing early) ----
        mask_sb = singles.tile([P, NKT], F32, tag="mask")
        ident_sb = singles.tile([P, P], F16, tag="ident")
        ones_sb = singles.tile([97, DH], F16, tag="ones")
        nc.vector.memset(ones_sb[:], 1.0)

        # ---- big SBUF-resident tensors ----
        qt_sb = big.tile([P, 4, L], F16, tag="qt")       # [p,t,l] = Qt[t*128+p, l]
        kt_sb = big.tile([P, 4, L], F16, tag="kt")
        vx_sb = big.tile([P, NKT, H, DH + 1], F16, tag="vx")  # V + mask column
        attnT = [
            big.tile([P, 4, QC], F16, tag=f"attnT{qc}", name=f"attnT{qc}")
            for qc in range(NQC)
        ]

        # ---- phase 1: projections (pools scoped so SBUF is reclaimed) ----
        proj_ctx = contextlib.ExitStack()
        stage = proj_ctx.enter_context(tc.tile_pool(name="stage", bufs=3))
        wpool = proj_ctx.enter_context(tc.tile_pool(name="wpool", bufs=3))

        def load_stage(src, eng):
            t = stage.tile([P, 2, L], F16, tag="stage")
            eng.dma_start(out=t[:], in_=src.rearrange("(t p) l -> p t l", p=P))
            return t

        def load_w(src, eng):
            t = wpool.tile([P, 2, DM], F16, tag="w")
            eng.dma_start(out=t[:], in_=src.rearrange("(t p) d -> p t d", p=P))
            return t

        # finest-gating loads first: the first Qt matmul needs only the t=0
        # halves of wq and q, so issue those as separate DMAs; v/wv and all
        # prefetch go after the Qt/Kt matmuls are emitted
        wq_sb = wpool.tile([P, 2, DM], F16, tag="w", name="wq_sb")
        q_sb = stage.tile([P, 2, L], F16, tag="stage", name="q_sb")
        wqr = wqT.rearrange("(t p) d -> p t d", p=P)
        qr = qT.rearrange("(t p) l -> p t l", p=P)
        for tt in range(2):
            nc.sync.dma_start(out=wq_sb[:, tt, :], in_=wqr[:, tt, :])
            nc.sync.dma_start(out=q_sb[:, tt, :], in_=qr[:, tt, :])
        k_sb, wk_sb = load_stage(kT, nc.sync), load_w(wkT, nc.sync)

        b_sb = [None] * H

        def load_bias(h):
            t = bpool.tile([P, NKT, L], F16, tag="bias")
            nc.sync.dma_start(out=t[:], in_=bT_r[:, h, :, :])
            b_sb[h] = t

        # Qt / Kt: out[m=dm-tile, n=l-chunk] = sum_din w?T[din, dm] * xT[din, l]
        # (bias/wo DMAs are emitted AFTER these matmuls so the 4MB+ of
        # prefetch doesn't compete with the loads that gate the first matmul)
        for x_sb, w_sb, dst in ((q_sb, wq_sb, qt_sb), (k_sb, wk_sb, kt_sb)):
            for mt in range(4):
                ps = spsum.tile([P, 2 * QC], F32, tag="sp", name="ps")
                for lc in range(NQC):
                    for kt2 in range(2):
                        nc.tensor.matmul(
                            ps[:, lc * QC : (lc + 1) * QC],
                            w_sb[:, kt2, mt * P : (mt + 1) * P],
                            x_sb[:, kt2, lc * QC : (lc + 1) * QC],
                            start=(kt2 == 0),
                            stop=(kt2 == 1),
                        )
                nc.vector.tensor_copy(out=dst[:, mt, :], in_=ps[:])

        v_sb, wv_sb = load_stage(vT, nc.sync), load_w(wvT, nc.sync)
        nc.sync.dma_start(out=mask_sb[:], in_=mask01[:])
        nc.sync.dma_start(out=ident_sb[:], in_=identI[:])
        wo_sb = singles.tile([P, 4, DM], F16, tag="wo")
        nc.sync.dma_start(out=wo_sb[:], in_=woT.rearrange("(t p) d -> p t d", p=P))
        load_bias(0)
        load_bias(1)

        # V: out[m=l-tile, n=dm] = sum_din vT[din, l] * wvT[din, dm]; mask rows
        for lt in range(NKT):
            ps = pvp.tile([P, DM], F32, tag="pv")
            for kt2 in range(2):
                nc.tensor.matmul(
                    ps[:],
                    v_sb[:, kt2, lt * P : (lt + 1) * P],
                    wv_sb[:, kt2, :],
                    start=(kt2 == 0),
                    stop=(kt2 == 1),
                )
            nc.vector.tensor_scalar_mul(
                out=vx_sb[:, lt, :, 0:DH],
                in0=ps.rearrange("p (h d) -> p h d", h=H),
                scalar1=mask_sb[:, lt : lt + 1],
            )
            # mask column (softmax denominator counts only unmasked keys)
            nc.vector.tensor_copy(
                out=vx_sb[:, lt, :, DH : DH + 1],
                in_=mask_sb[:, lt : lt + 1, None].to_broadcast((P, H, 1)),
            )

        proj_ctx.close()

        # ---- phase 2: attention, software-pipelined ----
        # Work queue of deferred closures (reciprocals, normalizations, WO).
        # One item pops at each flush point; emission order = engine order, so
        # deferred items land in the PE stream well after their DVE/DMA
        # producers have had time to run.
        pending = []

        def flush_one():
            if pending:
                item = pending.pop(0)
                if item is not None:
                    item()

        # per-qc staging: unnormalized PV (+denom row) for all 8 heads, and
        # denominator gather tiles at partitions {0,32,64} so one DVE
        # reciprocal serves 3 heads (128-lane parallelism needs the
        # denominators spread across partitions; {0,32,64} are the only legal
        # base partitions for the bcast matmul operands)
        HGRP = [(0, 1, 2), (3, 4, 5), (6, 7)]  # head groups per den tile
        NG = len(HGRP)
        pvsq = [
            big.tile([DH + 1, H, QC], F32, tag=f"pvq{qc}", name=f"pvq{qc}")
            for qc in range(NQC)
        ]
        dens = [
            big.tile([65, QC], F32, tag=f"den{i}", name=f"den{i}")
            for i in range(NG * NQC)
        ]
        recbs = [
            big.tile([65, QC], F16, tag=f"recb{i}", name=f"recb{i}")
            for i in range(NG * NQC)
        ]

        def emit_recip(i, last=False):
            def go():
                if last:
                    # tail path: ScalarE ln->exp(-x) reciprocal (~1.4us) beats
                    # the 3.35us DVE reciprocal when the kernel is draining
                    lntmp = small.tile([65, QC], F32, tag="lntmp")
                    nc.scalar.activation(
                        out=lntmp[:], in_=dens[i][:],
                        func=mybir.ActivationFunctionType.Ln,
                    )
                    nc.scalar.activation(
                        out=recbs[i][:], in_=lntmp[:],
                        func=mybir.ActivationFunctionType.Exp, scale=-1.0,
                    )
                else:
                    with nc.allow_low_precision(
                        reason="fp16 softmax recip matches kernel precision"
                    ):
                        nc.vector.reciprocal(out=recbs[i][:], in_=dens[i][:])

            pending.append(go)
            # pacing bubbles: the first dependent bcast pops ~2 flush points
            # (~4-5us of PE work) after the reciprocal is emitted
            pending.append(None)
            pending.append(None)

        def emit_norm(h, qc):
            ht, odd = h // 2, h % 2
            rp = 32 * (h % 3)  # partition of this head's denominator row
            recb = recbs[NG * qc + h // 3]

            def go():
                bps = bcp.tile([DH, QC], F32, tag="bps")
                nc.tensor.matmul(
                    bps[:],
                    ones_sb[rp : rp + 1, :],
                    recb[rp : rp + 1, :],
                    start=True,
                    stop=True,
                )
                if not odd:
                    nc.vector.tensor_mul(
                        out=attnT[qc][0:DH, ht, :],
                        in0=pvsq[qc][0:DH, h, :],
                        in1=bps[:],
                    )
                else:
                    tmp = small.tile([DH, QC], F16, tag="odd")
                    nc.vector.tensor_mul(
                        out=tmp[:], in0=pvsq[qc][0:DH, h, :], in1=bps[:]
                    )
                    nc.sync.dma_start(out=attnT[qc][DH:P, ht, :], in_=tmp[:])

            pending.append(go)

        def wo_chain(qc, ws, ltp, kts):
            for i in range(2):
                lt = 2 * ltp + i
                for kt4 in kts:
                    nc.tensor.matmul(
                        ws[:, i * QC : (i + 1) * QC],
                        attnT[qc][:, kt4, lt * P : (lt + 1) * P],
                        wo_sb[:, kt4, :],
                        start=(kt4 == 0),
                        stop=(kt4 == 3),
                        skip_group_check=True,
                    )

        def wo_evac(qc, ws, ltp):
            # alternate evac engine so the two 1us copies run in parallel
            ost = small.tile([P, 2 * QC], F32, tag="ost")
            if ltp == 0:
                nc.scalar.copy(out=ost[:], in_=ws[:])
            else:
                nc.vector.tensor_copy(out=ost[:], in_=ws[:])
            for i in range(2):
                lt = 2 * ltp + i
                nc.sync.dma_start(
                    out=out[qc * QC + lt * P : qc * QC + (lt + 1) * P, :],
                    in_=ost[:, i * QC : (i + 1) * QC],
                )

        def emit_wo(qc):
            def go():
                for ltp in range(2):  # pairs of 128-row output tiles
                    ws = spsum.tile([P, 2 * QC], F32, tag="sp", name="ws")
                    wo_chain(qc, ws, ltp, range(4))
                    wo_evac(qc, ws, ltp)

            pending.append(go)

        def emit_wo_split(qc):
            # final-qc WO: the first 3 dm-chunks only need heads 0-5, so they
            # run on the PE while the last head-group's reciprocal chain is
            # still in flight; the stop matmuls + evac follow the last norms
            ws_tiles = []

            def begin():
                for ltp in range(2):
                    ws = spsum.tile([P, 2 * QC], F32, tag="sp", name="ws")
                    wo_chain(qc, ws, ltp, range(3))
                    ws_tiles.append(ws)

            def end():
                for ltp in range(2):
                    wo_chain(qc, ws_tiles[ltp], ltp, [3])
                    wo_evac(qc, ws_tiles[ltp], ltp)

            return begin, end

        for h in range(H):
            hb = (h % 2) * DH  # partition base of head h inside its dm-tile
            ht = h // 2
            if h + 2 < H:
                load_bias(h + 2)
            bh = b_sb[h]
            for qc in range(NQC):
                qs = slice(qc * QC, (qc + 1) * QC)
                # -- scores: QK (start) + single identity bias matmul (stop) --
                ex = exps.tile([P, NKT, QC], F16, tag="ex")
                for ktp in range(NKT // 2):  # pairs of k-tiles share a psum
                    sp = spsum.tile([P, 2 * QC], F32, tag="sp")
                    for i in range(2):
                        kt = 2 * ktp + i
                        nc.tensor.matmul(
                            sp[:, i * QC : (i + 1) * QC],
                            kt_sb[hb : hb + DH, ht, kt * P : (kt + 1) * P],
                            qt_sb[hb : hb + DH, ht, qs],
                            start=True,
                            stop=False,
                        )
                    for i in range(2):
                        kt = 2 * ktp + i
                        nc.tensor.matmul(
                            sp[:, i * QC : (i + 1) * QC],
                            ident_sb[:],
                            bh[:, kt, qs],
                            start=False, stop=True, skip_group_check=True,
                        )
                    nc.scalar.activation(
                        out=ex[:, 2 * ktp : 2 * ktp + 2, :].rearrange(
                            "p a b -> p (a b)"
                        ),
                        in_=sp[:],
                        func=mybir.ActivationFunctionType.Exp,
                    )
                flush_one()
                # -- PV with appended mask column -> row 64 = denominator --
                pv = pvp.tile([DH + 1, QC], F32, tag="pv")
                for kt in range(NKT):
                    nc.tensor.matmul(
                        pv[:],
                        vx_sb[:, kt, h, :],
                        ex[:, kt, :],
                        start=(kt == 0),
                        stop=(kt == NKT - 1),
                    )
                # stage PV to SBUF (frees the PSUM bank early); gather the
                # denominator row into the group reciprocal tile at a
                # 32-stride partition via a tiny SBUF->SBUF DMA
                nc.vector.tensor_copy(out=pvsq[qc][:, h, :], in_=pv[:])
                nc.sync.dma_start(
                    out=dens[NG * qc + h // 3][32 * (h % 3) : 32 * (h % 3) + 1, :],
                    in_=pvsq[qc][DH : DH + 1, h, :],
                )
                if h in (2, 5, 7):
                    g = h // 3
                    final = h == H - 1 and qc == NQC - 1
                    emit_recip(NG * qc + g, last=final)
                    if final:
                        wo_begin, wo_end = emit_wo_split(qc)
                        pending.append(wo_begin)
                    for hh in HGRP[g]:
                        emit_norm(hh, qc)
                    if h == H - 1:
                        if final:
                            pending.append(wo_end)
                        else:
                            emit_wo(qc)
                flush_one()
                if len(pending) > 4:  # drain backlog so the end-tail is short
                    flush_one()

        while pending:
            flush_one()


def build_nc():
    from concourse import bacc

    nc = bacc.Bacc("TRN2", target_bir_lowering=False, debug=False)
    with tile.TileContext(nc) as tc:
        _emit(tc)
    nc.compile()
    return nc


_NC = None


def _get_nc():
    global _NC
    if _NC is None:
        _NC = build_nc()
    return _NC


def make_in_maps(queries, keys, values, attention_mask, adjacency_matrix,
                 distance_matrix, W_q, W_k, W_v, W_o, lambda_a, lambda_d):
    f = np.float32
    h16 = np.float16
    c = np.ascontiguousarray
    wqT = c((W_q.astype(f) * f(0.125)).T).astype(h16)
    wkT = c(W_k.astype(f).T).astype(h16)
    wvT = c(W_v.astype(f).T).astype(h16)
    woT = c(W_o.astype(f).T).astype(h16)
    identI = np.eye(P, dtype=h16)
    la = np.asarray(lambda_a, dtype=f)
    ld = np.asarray(lambda_d, dtype=f)
    in_maps = []
    for b in range(B):
        aT = c(np.asarray(adjacency_matrix[b], dtype=f).T)
        dT = c(np.asarray(distance_matrix[b], dtype=f).T)
        bT = np.empty((H, L, L), dtype=h16)
        for h in range(H):
            bT[h] = (la[h] * aT + ld[h] * dT).astype(h16)
        in_maps.append({
            "qT": c(queries[b].astype(f).T).astype(h16),
            "kT": c(keys[b].astype(f).T).astype(h16),
            "vT": c(values[b].astype(f).T).astype(h16),
            "wqT": wqT, "wkT": wkT, "wvT": wvT, "woT": woT,
            "bT": bT.reshape(H * L, L),
            "mask01": c((attention_mask[b] > 0).astype(f).reshape(NKT, P).T),
            "identI": identI,
        })
    return in_maps


def kernel(queries, keys, values, attention_mask, adjacency_matrix,
           distance_matrix, W_q, W_k, W_v, W_o, lambda_a, lambda_d, **kw):
    nc = _get_nc()
    in_maps = make_in_maps(queries, keys, values, attention_mask,
                           adjacency_matrix, distance_matrix,
                           W_q, W_k, W_v, W_o, lambda_a, lambda_d)
    res = run_bass_kernel_spmd(nc, in_maps, list(range(B)), **kw)
    outs = np.stack([res.results[i]["out"] for i in range(B)]).astype(np.float32)
    return outs


# revision 65
# speedup vs baseline: 1.0127x; 1.0127x over previous
"""MSRSA multi-head attention kernel for 8 Trainium2 NeuronCores.

Strategy: data-parallel over batch (B=8 -> 1 batch element per core).
Per core, for its batch element b:
  Qt = (W_q/8) @ queries^T        [512,1024]  (scale 1/8 folded into W_q)
  Kt = W_k @ keys^T               [512,1024]
  V  = values @ W_v^T             [1024,512]  (rows masked by attention_mask)
  per head h, scores are computed TRANSPOSED: S_T[k,q]:
     S_T = sum_d Kt[d,k]*Qt[d,q] + B_h^T[k,q]
  where B_h = la[h]*A + ld[h]*D is combined HOST-side per head (fp16) and
  injected into PSUM by a single identity matmul per k-tile (vs 2 scaled-
  identity matmuls in v1 -- halves bias PE cycles, 16MB/core DMA streamed
  per-head double-buffered).
  expS = exp(S_T) on ScalarE (PSUM -> SBUF evacuation is the exp)
  attnT_h[d,q] (+ denominator row) = sum_k V_ext[k, d|mask] * expS[k,q]
  (mask column of V_ext -> row 64 of PV output = softmax denominator)
  normalize via reciprocal_approx_fast (DVE) + K=1 ones-matmul broadcast;
  the broadcast matmul is EMITTED after the next head's QK block so the PE
  never idles waiting on the DVE reciprocal (also keeps the PE p-state at
  full clock).
  out = attnT contracted with W_o^T   [1024, 512]

Matmul operands are fp16 (1 PE cycle/row); accumulation fp32 in PSUM.
exp and softmax normalization run in fp32. Transposes host-side.
"""

import contextlib

import numpy as np

import concourse.bass as bass
import concourse.mybir as mybir
import concourse.tile as tile
from concourse.bass_utils import run_bass_kernel_spmd

B, L, DIN, DM, H = 8, 1024, 256, 512, 8
DH = DM // H  # 64
P = 128
NKT = L // P          # 8 k-tiles
NQC = 2               # q chunks
QC = L // NQC         # 512
F32 = mybir.dt.float32
F16 = mybir.dt.float16


def _emit(tc):
    nc = tc.nc

    def dram(name, shape, dtype=F16, kind="ExternalInput"):
        return nc.dram_tensor(name, shape, dtype, kind=kind).ap()

    qT = dram("qT", [DIN, L])
    kT = dram("kT", [DIN, L])
    vT = dram("vT", [DIN, L])
    wqT = dram("wqT", [DIN, DM])
    wkT = dram("wkT", [DIN, DM])
    wvT = dram("wvT", [DIN, DM])
    woT = dram("woT", [DM, DM])
    bT = dram("bT", [H * L, L])   # per-head combined bias, transposed
    identI = dram("identI", [P, P])
    mask01 = dram("mask01", [P, NKT], F32)
    out = dram("out", [L, DM], F32, kind="ExternalOutput")

    bT_r = bT.rearrange("(h t p) q -> p h t q", p=P, t=NKT)

    with contextlib.ExitStack() as ctx:
        singles = ctx.enter_context(tc.tile_pool(name="singles", bufs=1))
        big = ctx.enter_context(tc.tile_pool(name="big", bufs=1))
        bpool = ctx.enter_context(tc.tile_pool(name="bpool", bufs=3))
        exps = ctx.enter_context(tc.tile_pool(name="exps", bufs=3))
        small = ctx.enter_context(tc.tile_pool(name="small", bufs=2))
        spsum = ctx.enter_context(tc.tile_pool(name="spsum", bufs=2, space="PSUM"))
        pvp = ctx.enter_context(tc.tile_pool(name="pvp", bufs=2, space="PSUM"))
        bcp = ctx.enter_context(tc.tile_pool(name="bcp", bufs=2, space="PSUM"))

        # ---- small constants (DMAs issued late; they gate nothing early) ----
        mask_sb = singles.tile([P, NKT], F32, tag="mask")
        ident_sb = singles.tile([P, P], F16, tag="ident")
        ones_sb = singles.tile([97, DH], F16, tag="ones")
        nc.vector.memset(ones_sb[:], 1.0)

        # ---- big SBUF-resident tensors ----
        qt_sb = big.tile([P, 4, L], F16, tag="qt")       # [p,t,l] = Qt[t*128+p, l]
        kt_sb = big.tile([P, 4, L], F16, tag="kt")
        vx_sb = big.tile([P, NKT, H, DH + 1], F16, tag="vx")  # V + mask column
        attnT = [
            big.tile([P, 4, QC], F16, tag=f"attnT{qc}", name=f"attnT{qc}")
            for qc in range(NQC)
        ]

        # ---- phase 1: projections (pools scoped so SBUF is reclaimed) ----
        proj_ctx = contextlib.ExitStack()
        stage = proj_ctx.enter_context(tc.tile_pool(name="stage", bufs=3))
        wpool = proj_ctx.enter_context(tc.tile_pool(name="wpool", bufs=3))

        def load_stage(src, eng):
            t = stage.tile([P, 2, L], F16, tag="stage")
            eng.dma_start(out=t[:], in_=src.rearrange("(t p) l -> p t l", p=P))
            return t

        def load_w(src, eng):
            t = wpool.tile([P, 2, DM], F16, tag="w")
            eng.dma_start(out=t[:], in_=src.rearrange("(t p) d -> p t d", p=P))
            return t

        # finest-gating loads first: the first Qt matmul needs only the t=0
        # halves of wq and q, so issue those as separate DMAs; v/wv and all
        # prefetch go after the Qt/Kt matmuls are emitted
        wq_sb = wpool.tile([P, 2, DM], F16, tag="w", name="wq_sb")
        q_sb = stage.tile([P, 2, L], F16, tag="stage", name="q_sb")
        wqr = wqT.rearrange("(t p) d -> p t d", p=P)
        qr = qT.rearrange("(t p) l -> p t l", p=P)
        for tt in range(2):
            nc.sync.dma_start(out=wq_sb[:, tt, :], in_=wqr[:, tt, :])
            nc.sync.dma_start(out=q_sb[:, tt, :], in_=qr[:, tt, :])
        k_sb, wk_sb = load_stage(kT, nc.sync), load_w(wkT, nc.sync)

        b_sb = [None] * H

        def load_bias(h):
            t = bpool.tile([P, NKT, L], F16, tag="bias")
            nc.sync.dma_start(out=t[:], in_=bT_r[:, h, :, :])
            b_sb[h] = t

        # Qt / Kt: out[m=dm-tile, n=l-chunk] = sum_din w?T[din, dm] * xT[din, l]
        # (bias/wo DMAs are emitted AFTER these matmuls so the 4MB+ of
        # prefetch doesn't compete with the loads that gate the first matmul)
        for x_sb, w_sb, dst in ((q_sb, wq_sb, qt_sb), (k_sb, wk_sb, kt_sb)):
            for mt in range(4):
                ps = spsum.tile([P, 2 * QC], F32, tag="sp", name="ps")
                for lc in range(NQC):
                    for kt2 in range(2):
                        nc.tensor.matmul(
                            ps[:, lc * QC : (lc + 1) * QC],
                            w_sb[:, kt2, mt * P : (mt + 1) * P],
                            x_sb[:, kt2, lc * QC : (lc + 1) * QC],
                            start=(kt2 == 0),
                            stop=(kt2 == 1),
                        )
                nc.vector.tensor_copy(out=dst[:, mt, :], in_=ps[:])

        v_sb, wv_sb = load_stage(vT, nc.sync), load_w(wvT, nc.sync)
        nc.sync.dma_start(out=mask_sb[:], in_=mask01[:])
        nc.sync.dma_start(out=ident_sb[:], in_=identI[:])
        wo_sb = singles.tile([P, 4, DM], F16, tag="wo")
        nc.sync.dma_start(out=wo_sb[:], in_=woT.rearrange("(t p) d -> p t d", p=P))
        load_bias(0)
        load_bias(1)

        # V: out[m=l-tile, n=dm] = sum_din vT[din, l] * wvT[din, dm]; mask rows
        for lt in range(NKT):
            ps = pvp.tile([P, DM], F32, tag="pv")
            for kt2 in range(2):
                nc.tensor.matmul(
                    ps[:],
                    v_sb[:, kt2, lt * P : (lt + 1) * P],
                    wv_sb[:, kt2, :],
                    start=(kt2 == 0),
                    stop=(kt2 == 1),
                )
            nc.vector.tensor_scalar_mul(
                out=vx_sb[:, lt, :, 0:DH],
                in0=ps.rearrange("p (h d) -> p h d", h=H),
                scalar1=mask_sb[:, lt : lt + 1],
            )
            # mask column (softmax denominator counts only unmasked keys)
            nc.vector.tensor_copy(
                out=vx_sb[:, lt, :, DH : DH + 1],
                in_=mask_sb[:, lt : lt + 1, None].to_broadcast((P, H, 1)),
            )

        proj_ctx.close()

        # ---- phase 2: attention, software-pipelined ----
        # Work queue of deferred closures (reciprocals, normalizations, WO).
        # One item pops at each flush point; emission order = engine order, so
        # deferred items land in the PE stream well after their DVE/DMA
        # producers have had time to run.
        pending = []

        def flush_one():
            if pending:
                item = pending.pop(0)
                if item is not None:
                    item()

        # per-qc staging: unnormalized PV (+denom row) for all 8 heads, and
        # denominator gather tiles at partitions {0,32,64} so one DVE
        # reciprocal serves 3 heads (128-lane parallelism needs the
        # denominators spread across partitions; {0,32,64} are the only legal
        # base partitions for the bcast matmul operands)
        HGRP = [(0, 1, 2), (3, 4, 5), (6, 7)]  # head groups per den tile
        NG = len(HGRP)
        pvsq = [
            big.tile([DH + 1, H, QC], F32, tag=f"pvq{qc}", name=f"pvq{qc}")
            for qc in range(NQC)
        ]
        dens = [
            big.tile([65, QC], F32, tag=f"den{i}", name=f"den{i}")
            for i in range(NG * NQC)
        ]
        recbs = [
            big.tile([65, QC], F16, tag=f"recb{i}", name=f"recb{i}")
            for i in range(NG * NQC)
        ]

        def emit_recip(i, last=False):
            def go():
                with nc.allow_low_precision(
                    reason="fp16 softmax recip matches kernel precision"
                ):
                    nc.vector.reciprocal(out=recbs[i][:], in_=dens[i][:])

            pending.append(go)
            # pacing bubbles: the first dependent bcast pops ~2 flush points
            # (~4-5us of PE work) after the reciprocal is emitted
            pending.append(None)
            pending.append(None)

        def emit_norm(h, qc):
            ht, odd = h // 2, h % 2
            rp = 32 * (h % 3)  # partition of this head's denominator row
            recb = recbs[NG * qc + h // 3]

            def go():
                bps = bcp.tile([DH, QC], F32, tag="bps")
                nc.tensor.matmul(
                    bps[:],
                    ones_sb[rp : rp + 1, :],
                    recb[rp : rp + 1, :],
                    start=True,
                    stop=True,
                )
                if not odd:
                    nc.vector.tensor_mul(
                        out=attnT[qc][0:DH, ht, :],
                        in0=pvsq[qc][0:DH, h, :],
                        in1=bps[:],
                    )
                else:
                    tmp = small.tile([DH, QC], F16, tag="odd")
                    nc.vector.tensor_mul(
                        out=tmp[:], in0=pvsq[qc][0:DH, h, :], in1=bps[:]
                    )
                    nc.sync.dma_start(out=attnT[qc][DH:P, ht, :], in_=tmp[:])

            pending.append(go)

        def wo_chain(qc, ws, ltp, kts):
            for i in range(2):
                lt = 2 * ltp + i
                for kt4 in kts:
                    nc.tensor.matmul(
                        ws[:, i * QC : (i + 1) * QC],
                        attnT[qc][:, kt4, lt * P : (lt + 1) * P],
                        wo_sb[:, kt4, :],
                        start=(kt4 == 0),
                        stop=(kt4 == 3),
                        skip_group_check=True,
                    )

        def wo_evac(qc, ws, ltp):
            # alternate evac engine so the two 1us copies run in parallel
            ost = small.tile([P, 2 * QC], F32, tag="ost")
            if ltp == 0:
                nc.scalar.copy(out=ost[:], in_=ws[:])
            else:
                nc.vector.tensor_copy(out=ost[:], in_=ws[:])
            for i in range(2):
                lt = 2 * ltp + i
                nc.sync.dma_start(
                    out=out[qc * QC + lt * P : qc * QC + (lt + 1) * P, :],
                    in_=ost[:, i * QC : (i + 1) * QC],
                )

        def emit_wo(qc):
            def go():
                for ltp in range(2):  # pairs of 128-row output tiles
                    ws = spsum.tile([P, 2 * QC], F32, tag="sp", name="ws")
                    wo_chain(qc, ws, ltp, range(4))
                    wo_evac(qc, ws, ltp)

            pending.append(go)

        def emit_wo_split(qc):
            # final-qc WO: the first 3 dm-chunks only need heads 0-5, so they
            # run on the PE while the last head-group's reciprocal chain is
            # still in flight; the stop matmuls + evac follow the last norms
            ws_tiles = []

            def begin():
                for ltp in range(2):
                    ws = spsum.tile([P, 2 * QC], F32, tag="sp", name="ws")
                    wo_chain(qc, ws, ltp, range(3))
                    ws_tiles.append(ws)

            def end():
                for ltp in range(2):
                    wo_chain(qc, ws_tiles[ltp], ltp, [3])
                    wo_evac(qc, ws_tiles[ltp], ltp)

            return begin, end

        for h in range(H):
            hb = (h % 2) * DH  # partition base of head h inside its dm-tile
            ht = h // 2
            if h + 2 < H:
                load_bias(h + 2)
            bh = b_sb[h]
            for qc in range(NQC):
                qs = slice(qc * QC, (qc + 1) * QC)
                # -- scores: QK (start) + single identity bias matmul (stop) --
                ex = exps.tile([P, NKT, QC], F16, tag="ex")
                for ktp in range(NKT // 2):  # pairs of k-tiles share a psum
                    sp = spsum.tile([P, 2 * QC], F32, tag="sp")
                    for i in range(2):
                        kt = 2 * ktp + i
                        nc.tensor.matmul(
                            sp[:, i * QC : (i + 1) * QC],
                            kt_sb[hb : hb + DH, ht, kt * P : (kt + 1) * P],
                            qt_sb[hb : hb + DH, ht, qs],
                            start=True,
                            stop=False,
                        )
                    for i in range(2):
                        kt = 2 * ktp + i
                        nc.tensor.matmul(
                            sp[:, i * QC : (i + 1) * QC],
                            ident_sb[:],
                            bh[:, kt, qs],
                            start=False, stop=True, skip_group_check=True,
                        )
                    nc.scalar.activation(
                        out=ex[:, 2 * ktp : 2 * ktp + 2, :].rearrange(
                            "p a b -> p (a b)"
                        ),
                        in_=sp[:],
                        func=mybir.ActivationFunctionType.Exp,
                    )
                flush_one()
                # -- PV with appended mask column -> row 64 = denominator --
                pv = pvp.tile([DH + 1, QC], F32, tag="pv")
                for kt in range(NKT):
                    nc.tensor.matmul(
                        pv[:],
                        vx_sb[:, kt, h, :],
                        ex[:, kt, :],
                        start=(kt == 0),
                        stop=(kt == NKT - 1),
                    )
                # stage PV to SBUF (frees the PSUM bank early); gather the
                # denominator row into the group reciprocal tile at a
                # 32-stride partition via a tiny SBUF->SBUF DMA
                nc.vector.tensor_copy(out=pvsq[qc][:, h, :], in_=pv[:])
                nc.sync.dma_start(
                    out=dens[NG * qc + h // 3][32 * (h % 3) : 32 * (h % 3) + 1, :],
                    in_=pvsq[qc][DH : DH + 1, h, :],
                )
                if h in (2, 5, 7):
                    g = h // 3
                    final = h == H - 1 and qc == NQC - 1
                    emit_recip(NG * qc + g, last=final)
                    if final:
                        wo_begin, wo_end = emit_wo_split(qc)
                        pending.append(wo_begin)
                    for hh in HGRP[g]:
                        emit_norm(hh, qc)
                    if h == H - 1:
                        if final:
                            pending.append(wo_end)
                        else:
                            emit_wo(qc)
                flush_one()
                if len(pending) > 4:  # drain backlog so the end-tail is short
                    flush_one()

        while pending:
            flush_one()


def build_nc():
    from concourse import bacc

    nc = bacc.Bacc("TRN2", target_bir_lowering=False, debug=False)
    with tile.TileContext(nc) as tc:
        _emit(tc)
    nc.compile()
    return nc


_NC = None


def _get_nc():
    global _NC
    if _NC is None:
        _NC = build_nc()
    return _NC


def make_in_maps(queries, keys, values, attention_mask, adjacency_matrix,
                 distance_matrix, W_q, W_k, W_v, W_o, lambda_a, lambda_d):
    f = np.float32
    h16 = np.float16
    c = np.ascontiguousarray
    wqT = c((W_q.astype(f) * f(0.125)).T).astype(h16)
    wkT = c(W_k.astype(f).T).astype(h16)
    wvT = c(W_v.astype(f).T).astype(h16)
    woT = c(W_o.astype(f).T).astype(h16)
    identI = np.eye(P, dtype=h16)
    la = np.asarray(lambda_a, dtype=f)
    ld = np.asarray(lambda_d, dtype=f)
    in_maps = []
    for b in range(B):
        aT = c(np.asarray(adjacency_matrix[b], dtype=f).T)
        dT = c(np.asarray(distance_matrix[b], dtype=f).T)
        bT = np.empty((H, L, L), dtype=h16)
        for h in range(H):
            bT[h] = (la[h] * aT + ld[h] * dT).astype(h16)
        in_maps.append({
            "qT": c(queries[b].astype(f).T).astype(h16),
            "kT": c(keys[b].astype(f).T).astype(h16),
            "vT": c(values[b].astype(f).T).astype(h16),
            "wqT": wqT, "wkT": wkT, "wvT": wvT, "woT": woT,
            "bT": bT.reshape(H * L, L),
            "mask01": c((attention_mask[b] > 0).astype(f).reshape(NKT, P).T),
            "identI": identI,
        })
    return in_maps


def kernel(queries, keys, values, attention_mask, adjacency_matrix,
           distance_matrix, W_q, W_k, W_v, W_o, lambda_a, lambda_d, **kw):
    nc = _get_nc()
    in_maps = make_in_maps(queries, keys, values, attention_mask,
                           adjacency_matrix, distance_matrix,
                           W_q, W_k, W_v, W_o, lambda_a, lambda_d)
    res = run_bass_kernel_spmd(nc, in_maps, list(range(B)), **kw)
    outs = np.stack([res.results[i]["out"] for i in range(B)]).astype(np.float32)
    return outs


# revision 66
# speedup vs baseline: 1.0131x; 1.0004x over previous
"""MSRSA multi-head attention kernel for 8 Trainium2 NeuronCores.

Strategy: data-parallel over batch (B=8 -> 1 batch element per core).
Per core, for its batch element b:
  Qt = (W_q/8) @ queries^T        [512,1024]  (scale 1/8 folded into W_q)
  Kt = W_k @ keys^T               [512,1024]
  V  = values @ W_v^T             [1024,512]  (rows masked by attention_mask)
  per head h, scores are computed TRANSPOSED: S_T[k,q]:
     S_T = sum_d Kt[d,k]*Qt[d,q] + B_h^T[k,q]
  where B_h = la[h]*A + ld[h]*D is combined HOST-side per head (fp16) and
  injected into PSUM by a single identity matmul per k-tile (vs 2 scaled-
  identity matmuls in v1 -- halves bias PE cycles, 16MB/core DMA streamed
  per-head double-buffered).
  expS = exp(S_T) on ScalarE (PSUM -> SBUF evacuation is the exp)
  attnT_h[d,q] (+ denominator row) = sum_k V_ext[k, d|mask] * expS[k,q]
  (mask column of V_ext -> row 64 of PV output = softmax denominator)
  normalize via reciprocal_approx_fast (DVE) + K=1 ones-matmul broadcast;
  the broadcast matmul is EMITTED after the next head's QK block so the PE
  never idles waiting on the DVE reciprocal (also keeps the PE p-state at
  full clock).
  out = attnT contracted with W_o^T   [1024, 512]

Matmul operands are fp16 (1 PE cycle/row); accumulation fp32 in PSUM.
exp and softmax normalization run in fp32. Transposes host-side.
"""

import contextlib

import numpy as np

import concourse.bass as bass
import concourse.mybir as mybir
import concourse.tile as tile
from concourse.bass_utils import run_bass_kernel_spmd

B, L, DIN, DM, H = 8, 1024, 256, 512, 8
DH = DM // H  # 64
P = 128
NKT = L // P          # 8 k-tiles
NQC = 2               # q chunks
QC = L // NQC         # 512
F32 = mybir.dt.float32
F16 = mybir.dt.float16


def _emit(tc):
    nc = tc.nc

    def dram(name, shape, dtype=F16, kind="ExternalInput"):
        return nc.dram_tensor(name, shape, dtype, kind=kind).ap()

    qT = dram("qT", [DIN, L])
    kT = dram("kT", [DIN, L])
    vT = dram("vT", [DIN, L])
    wqT = dram("wqT", [DIN, DM])
    wkT = dram("wkT", [DIN, DM])
    wvT = dram("wvT", [DIN, DM])
    woT = dram("woT", [DM, DM])
    bT = dram("bT", [H * L, L])   # per-head combined bias, transposed
    identI = dram("identI", [P, P])
    mask01 = dram("mask01", [P, NKT], F32)
    out = dram("out", [L, DM], F32, kind="ExternalOutput")

    bT_r = bT.rearrange("(h t p) q -> p h t q", p=P, t=NKT)

    with contextlib.ExitStack() as ctx:
        singles = ctx.enter_context(tc.tile_pool(name="singles", bufs=1))
        big = ctx.enter_context(tc.tile_pool(name="big", bufs=1))
        bpool = ctx.enter_context(tc.tile_pool(name="bpool", bufs=3))
        exps = ctx.enter_context(tc.tile_pool(name="exps", bufs=3))
        small = ctx.enter_context(tc.tile_pool(name="small", bufs=2))
        spsum = ctx.enter_context(tc.tile_pool(name="spsum", bufs=2, space="PSUM"))
        pvp = ctx.enter_context(tc.tile_pool(name="pvp", bufs=2, space="PSUM"))
        bcp = ctx.enter_context(tc.tile_pool(name="bcp", bufs=2, space="PSUM"))

        # ---- small constants (DMAs issued late; they gate nothing early) ----
        mask_sb = singles.tile([P, NKT], F32, tag="mask")
        ident_sb = singles.tile([P, P], F16, tag="ident")
        ones_sb = singles.tile([97, DH], F16, tag="ones")
        nc.vector.memset(ones_sb[:], 1.0)

        # ---- big SBUF-resident tensors ----
        qt_sb = big.tile([P, 4, L], F16, tag="qt")       # [p,t,l] = Qt[t*128+p, l]
        kt_sb = big.tile([P, 4, L], F16, tag="kt")
        vx_sb = big.tile([P, NKT, H, DH + 1], F16, tag="vx")  # V + mask column
        attnT = [
            big.tile([P, 4, QC], F16, tag=f"attnT{qc}", name=f"attnT{qc}")
            for qc in range(NQC)
        ]

        # ---- phase 1: projections (pools scoped so SBUF is reclaimed) ----
        proj_ctx = contextlib.ExitStack()
        stage = proj_ctx.enter_context(tc.tile_pool(name="stage", bufs=3))
        wpool = proj_ctx.enter_context(tc.tile_pool(name="wpool", bufs=3))

        def load_stage(src, eng):
            t = stage.tile([P, 2, L], F16, tag="stage")
            eng.dma_start(out=t[:], in_=src.rearrange("(t p) l -> p t l", p=P))
            return t

        def load_w(src, eng):
            t = wpool.tile([P, 2, DM], F16, tag="w")
            eng.dma_start(out=t[:], in_=src.rearrange("(t p) d -> p t d", p=P))
            return t

        # finest-gating loads first: the first Qt matmul needs only the t=0
        # halves of wq and q, so issue those as separate DMAs; v/wv and all
        # prefetch go after the Qt/Kt matmuls are emitted
        wq_sb = wpool.tile([P, 2, DM], F16, tag="w", name="wq_sb")
        q_sb = stage.tile([P, 2, L], F16, tag="stage", name="q_sb")
        wqr = wqT.rearrange("(t p) d -> p t d", p=P)
        qr = qT.rearrange("(t p) l -> p t l", p=P)
        for tt in range(2):
            nc.sync.dma_start(out=wq_sb[:, tt, :], in_=wqr[:, tt, :])
            nc.sync.dma_start(out=q_sb[:, tt, :], in_=qr[:, tt, :])
        k_sb, wk_sb = load_stage(kT, nc.sync), load_w(wkT, nc.sync)

        b_sb = [None] * H

        def load_bias(h):
            t = bpool.tile([P, NKT, L], F16, tag="bias")
            nc.sync.dma_start(out=t[:], in_=bT_r[:, h, :, :])
            b_sb[h] = t

        # Qt / Kt: out[m=dm-tile, n=l-chunk] = sum_din w?T[din, dm] * xT[din, l]
        # (bias/wo DMAs are emitted AFTER these matmuls so the 4MB+ of
        # prefetch doesn't compete with the loads that gate the first matmul)
        for x_sb, w_sb, dst in ((q_sb, wq_sb, qt_sb), (k_sb, wk_sb, kt_sb)):
            for mt in range(4):
                ps = spsum.tile([P, 2 * QC], F32, tag="sp", name="ps")
                for lc in range(NQC):
                    for kt2 in range(2):
                        nc.tensor.matmul(
                            ps[:, lc * QC : (lc + 1) * QC],
                            w_sb[:, kt2, mt * P : (mt + 1) * P],
                            x_sb[:, kt2, lc * QC : (lc + 1) * QC],
                            start=(kt2 == 0),
                            stop=(kt2 == 1),
                        )
                nc.vector.tensor_copy(out=dst[:, mt, :], in_=ps[:])

        v_sb, wv_sb = load_stage(vT, nc.sync), load_w(wvT, nc.sync)
        nc.sync.dma_start(out=mask_sb[:], in_=mask01[:])
        nc.sync.dma_start(out=ident_sb[:], in_=identI[:])
        wo_sb = singles.tile([P, 4, DM], F16, tag="wo")
        nc.sync.dma_start(out=wo_sb[:], in_=woT.rearrange("(t p) d -> p t d", p=P))
        load_bias(0)
        load_bias(1)

        # V: out[m=l-tile, n=dm] = sum_din vT[din, l] * wvT[din, dm]; mask rows
        for lt in range(NKT):
            ps = pvp.tile([P, DM], F32, tag="pv")
            for kt2 in range(2):
                nc.tensor.matmul(
                    ps[:],
                    v_sb[:, kt2, lt * P : (lt + 1) * P],
                    wv_sb[:, kt2, :],
                    start=(kt2 == 0),
                    stop=(kt2 == 1),
                )
            nc.vector.tensor_scalar_mul(
                out=vx_sb[:, lt, :, 0:DH],
                in0=ps.rearrange("p (h d) -> p h d", h=H),
                scalar1=mask_sb[:, lt : lt + 1],
            )
            # mask column (softmax denominator counts only unmasked keys)
            nc.vector.tensor_copy(
                out=vx_sb[:, lt, :, DH : DH + 1],
                in_=mask_sb[:, lt : lt + 1, None].to_broadcast((P, H, 1)),
            )

        proj_ctx.close()

        # ---- phase 2: attention, software-pipelined ----
        # Work queue of deferred closures (reciprocals, normalizations, WO).
        # One item pops at each flush point; emission order = engine order, so
        # deferred items land in the PE stream well after their DVE/DMA
        # producers have had time to run.
        pending = []

        def flush_one():
            if pending:
                item = pending.pop(0)
                if item is not None:
                    item()

        # per-qc staging: unnormalized PV (+denom row) for all 8 heads, and
        # denominator gather tiles at partitions {0,32,64} so one DVE
        # reciprocal serves 3 heads (128-lane parallelism needs the
        # denominators spread across partitions; {0,32,64} are the only legal
        # base partitions for the bcast matmul operands)
        HGRP = [(0, 1, 2), (3, 4, 5), (6, 7)]  # head groups per den tile
        NG = len(HGRP)
        pvsq = [
            big.tile([DH + 1, H, QC], F32, tag=f"pvq{qc}", name=f"pvq{qc}")
            for qc in range(NQC)
        ]
        dens = [
            big.tile([65, QC], F32, tag=f"den{i}", name=f"den{i}")
            for i in range(NG * NQC)
        ]
        recbs = [
            big.tile([65, QC], F16, tag=f"recb{i}", name=f"recb{i}")
            for i in range(NG * NQC)
        ]

        def emit_recip(i, last=False):
            def go():
                with nc.allow_low_precision(
                    reason="fp16 softmax recip matches kernel precision"
                ):
                    nc.vector.reciprocal(out=recbs[i][:], in_=dens[i][:])

            pending.append(go)
            # pacing bubbles: the first dependent bcast pops ~2 flush points
            # (~4-5us of PE work) after the reciprocal is emitted
            pending.append(None)
            pending.append(None)

        def emit_norm(h, qc):
            ht, odd = h // 2, h % 2
            rp = 32 * (h % 3)  # partition of this head's denominator row
            recb = recbs[NG * qc + h // 3]

            def go():
                bps = bcp.tile([DH, QC], F32, tag="bps")
                nc.tensor.matmul(
                    bps[:],
                    ones_sb[rp : rp + 1, :],
                    recb[rp : rp + 1, :],
                    start=True,
                    stop=True,
                )
                if not odd:
                    nc.vector.tensor_mul(
                        out=attnT[qc][0:DH, ht, :],
                        in0=pvsq[qc][0:DH, h, :],
                        in1=bps[:],
                    )
                else:
                    tmp = small.tile([DH, QC], F16, tag="odd")
                    nc.vector.tensor_mul(
                        out=tmp[:], in0=pvsq[qc][0:DH, h, :], in1=bps[:]
                    )
                    nc.sync.dma_start(out=attnT[qc][DH:P, ht, :], in_=tmp[:])

            pending.append(go)

        def wo_chain(qc, ws, ltp, kts):
            for i in range(2):
                lt = 2 * ltp + i
                for kt4 in kts:
                    nc.tensor.matmul(
                        ws[:, i * QC : (i + 1) * QC],
                        attnT[qc][:, kt4, lt * P : (lt + 1) * P],
                        wo_sb[:, kt4, :],
                        start=(kt4 == 0),
                        stop=(kt4 == 3),
                        skip_group_check=True,
                    )

        def wo_evac(qc, ws, ltp):
            # alternate evac engine so the two 1us copies run in parallel
            ost = small.tile([P, 2 * QC], F32, tag="ost")
            if ltp == 0:
                nc.scalar.copy(out=ost[:], in_=ws[:])
            else:
                nc.vector.tensor_copy(out=ost[:], in_=ws[:])
            for i in range(2):
                lt = 2 * ltp + i
                nc.sync.dma_start(
                    out=out[qc * QC + lt * P : qc * QC + (lt + 1) * P, :],
                    in_=ost[:, i * QC : (i + 1) * QC],
                )

        def emit_wo(qc):
            def go():
                for ltp in range(2):  # pairs of 128-row output tiles
                    ws = spsum.tile([P, 2 * QC], F32, tag="sp", name="ws")
                    wo_chain(qc, ws, ltp, range(4))
                    wo_evac(qc, ws, ltp)

            pending.append(go)

        def emit_wo_split(qc):
            # final-qc WO: the first 3 dm-chunks only need heads 0-5, so they
            # run on the PE while the last head-group's reciprocal chain is
            # still in flight; the stop matmuls + evac follow the last norms
            ws_tiles = []

            def begin():
                for ltp in range(2):
                    ws = spsum.tile([P, 2 * QC], F32, tag="sp", name="ws")
                    wo_chain(qc, ws, ltp, range(3))
                    ws_tiles.append(ws)

            def end():
                for ltp in range(2):
                    wo_chain(qc, ws_tiles[ltp], ltp, [3])
                    wo_evac(qc, ws_tiles[ltp], ltp)

            return begin, end

        for h in range(H):
            hb = (h % 2) * DH  # partition base of head h inside its dm-tile
            ht = h // 2
            if h + 2 < H:
                load_bias(h + 2)
            bh = b_sb[h]
            for qc in range(NQC):
                qs = slice(qc * QC, (qc + 1) * QC)
                # -- scores: QK (start) + single identity bias matmul (stop) --
                ex = exps.tile([P, NKT, QC], F16, tag="ex")
                for ktp in range(NKT // 2):  # pairs of k-tiles share a psum
                    sp = spsum.tile([P, 2 * QC], F32, tag="sp")
                    for i in range(2):
                        kt = 2 * ktp + i
                        nc.tensor.matmul(
                            sp[:, i * QC : (i + 1) * QC],
                            kt_sb[hb : hb + DH, ht, kt * P : (kt + 1) * P],
                            qt_sb[hb : hb + DH, ht, qs],
                            start=True,
                            stop=False,
                        )
                    for i in range(2):
                        kt = 2 * ktp + i
                        nc.tensor.matmul(
                            sp[:, i * QC : (i + 1) * QC],
                            ident_sb[:],
                            bh[:, kt, qs],
                            start=False, stop=True, skip_group_check=True,
                        )
                    nc.scalar.activation(
                        out=ex[:, 2 * ktp : 2 * ktp + 2, :].rearrange(
                            "p a b -> p (a b)"
                        ),
                        in_=sp[:],
                        func=mybir.ActivationFunctionType.Exp,
                    )
                flush_one()
                # -- PV with appended mask column -> row 64 = denominator --
                pv = pvp.tile([DH + 1, QC], F32, tag="pv")
                for kt in range(NKT):
                    nc.tensor.matmul(
                        pv[:],
                        vx_sb[:, kt, h, :],
                        ex[:, kt, :],
                        start=(kt == 0),
                        stop=(kt == NKT - 1),
                    )
                # stage PV to SBUF (frees the PSUM bank early); gather the
                # denominator row into the group reciprocal tile at a
                # 32-stride partition via a tiny SBUF->SBUF DMA
                nc.vector.tensor_copy(out=pvsq[qc][:, h, :], in_=pv[:])
                nc.sync.dma_start(
                    out=dens[NG * qc + h // 3][32 * (h % 3) : 32 * (h % 3) + 1, :],
                    in_=pvsq[qc][DH : DH + 1, h, :],
                )
                if h in (2, 5, 7):
                    g = h // 3
                    final = h == H - 1 and qc == NQC - 1
                    emit_recip(NG * qc + g, last=final)
                    if final:
                        wo_begin, wo_end = emit_wo_split(qc)
                        pending.append(wo_begin)
                    for hh in HGRP[g]:
                        emit_norm(hh, qc)
                    if h == H - 1:
                        if final:
                            pending.append(wo_end)
                        else:
                            emit_wo(qc)
                flush_one()

        while pending:
            flush_one()


def build_nc():
    from concourse import bacc

    nc = bacc.Bacc("TRN2", target_bir_lowering=False, debug=False)
    with tile.TileContext(nc) as tc:
        _emit(tc)
    nc.compile()
    return nc


_NC = None


def _get_nc():
    global _NC
    if _NC is None:
        _NC = build_nc()
    return _NC


def make_in_maps(queries, keys, values, attention_mask, adjacency_matrix,
                 distance_matrix, W_q, W_k, W_v, W_o, lambda_a, lambda_d):
    f = np.float32
    h16 = np.float16
    c = np.ascontiguousarray
    wqT = c((W_q.astype(f) * f(0.125)).T).astype(h16)
    wkT = c(W_k.astype(f).T).astype(h16)
    wvT = c(W_v.astype(f).T).astype(h16)
    woT = c(W_o.astype(f).T).astype(h16)
    identI = np.eye(P, dtype=h16)
    la = np.asarray(lambda_a, dtype=f)
    ld = np.asarray(lambda_d, dtype=f)
    in_maps = []
    for b in range(B):
        aT = c(np.asarray(adjacency_matrix[b], dtype=f).T)
        dT = c(np.asarray(distance_matrix[b], dtype=f).T)
        bT = np.empty((H, L, L), dtype=h16)
        for h in range(H):
            bT[h] = (la[h] * aT + ld[h] * dT).astype(h16)
        in_maps.append({
            "qT": c(queries[b].astype(f).T).astype(h16),
            "kT": c(keys[b].astype(f).T).astype(h16),
            "vT": c(values[b].astype(f).T).astype(h16),
            "wqT": wqT, "wkT": wkT, "wvT": wvT, "woT": woT,
            "bT": bT.reshape(H * L, L),
            "mask01": c((attention_mask[b] > 0).astype(f).reshape(NKT, P).T),
            "identI": identI,
        })
    return in_maps


def kernel(queries, keys, values, attention_mask, adjacency_matrix,
           distance_matrix, W_q, W_k, W_v, W_o, lambda_a, lambda_d, **kw):
    nc = _get_nc()
    in_maps = make_in_maps(queries, keys, values, attention_mask,
                           adjacency_matrix, distance_matrix,
                           W_q, W_k, W_v, W_o, lambda_a, lambda_d)
    res = run_bass_kernel_spmd(nc, in_maps, list(range(B)), **kw)
    outs = np.stack([res.results[i]["out"] for i in range(B)]).astype(np.float32)
    return outs


# revision 67
# speedup vs baseline: 1.0165x; 1.0034x over previous
"""MSRSA multi-head attention kernel for 8 Trainium2 NeuronCores.

Strategy: data-parallel over batch (B=8 -> 1 batch element per core).
Per core, for its batch element b:
  Qt = (W_q/8) @ queries^T        [512,1024]  (scale 1/8 folded into W_q)
  Kt = W_k @ keys^T               [512,1024]
  V  = values @ W_v^T             [1024,512]  (rows masked by attention_mask)
  per head h, scores are computed TRANSPOSED: S_T[k,q]:
     S_T = sum_d Kt[d,k]*Qt[d,q] + B_h^T[k,q]
  where B_h = la[h]*A + ld[h]*D is combined HOST-side per head (fp16) and
  injected into PSUM by a single identity matmul per k-tile (vs 2 scaled-
  identity matmuls in v1 -- halves bias PE cycles, 16MB/core DMA streamed
  per-head double-buffered).
  expS = exp(S_T) on ScalarE (PSUM -> SBUF evacuation is the exp)
  attnT_h[d,q] (+ denominator row) = sum_k V_ext[k, d|mask] * expS[k,q]
  (mask column of V_ext -> row 64 of PV output = softmax denominator)
  normalize via reciprocal_approx_fast (DVE) + K=1 ones-matmul broadcast;
  the broadcast matmul is EMITTED after the next head's QK block so the PE
  never idles waiting on the DVE reciprocal (also keeps the PE p-state at
  full clock).
  out = attnT contracted with W_o^T   [1024, 512]

Matmul operands are fp16 (1 PE cycle/row); accumulation fp32 in PSUM.
exp and softmax normalization run in fp32. Transposes host-side.
"""

import contextlib

import numpy as np

import concourse.bass as bass
import concourse.mybir as mybir
import concourse.tile as tile
from concourse.bass_utils import run_bass_kernel_spmd

B, L, DIN, DM, H = 8, 1024, 256, 512, 8
DH = DM // H  # 64
P = 128
NKT = L // P          # 8 k-tiles
NQC = 2               # q chunks
QC = L // NQC         # 512
F32 = mybir.dt.float32
F16 = mybir.dt.float16


def _emit(tc):
    nc = tc.nc

    def dram(name, shape, dtype=F16, kind="ExternalInput"):
        return nc.dram_tensor(name, shape, dtype, kind=kind).ap()

    qT = dram("qT", [DIN, L])
    kT = dram("kT", [DIN, L])
    vT = dram("vT", [DIN, L])
    wqT = dram("wqT", [DIN, DM])
    wkT = dram("wkT", [DIN, DM])
    wvT = dram("wvT", [DIN, DM])
    woT = dram("woT", [DM, DM])
    bT = dram("bT", [H * L, L])   # per-head combined bias, transposed
    identI = dram("identI", [P, P])
    mask01 = dram("mask01", [P, NKT], F32)
    out = dram("out", [L, DM], F32, kind="ExternalOutput")

    bT_r = bT.rearrange("(h t p) q -> p h t q", p=P, t=NKT)

    with contextlib.ExitStack() as ctx:
        singles = ctx.enter_context(tc.tile_pool(name="singles", bufs=1))
        big = ctx.enter_context(tc.tile_pool(name="big", bufs=1))
        bpool = ctx.enter_context(tc.tile_pool(name="bpool", bufs=3))
        exps = ctx.enter_context(tc.tile_pool(name="exps", bufs=3))
        small = ctx.enter_context(tc.tile_pool(name="small", bufs=2))
        spsum = ctx.enter_context(tc.tile_pool(name="spsum", bufs=2, space="PSUM"))
        pvp = ctx.enter_context(tc.tile_pool(name="pvp", bufs=2, space="PSUM"))
        bcp = ctx.enter_context(tc.tile_pool(name="bcp", bufs=2, space="PSUM"))

        # ---- small constants (DMAs issued late; they gate nothing early) ----
        mask_sb = singles.tile([P, NKT], F32, tag="mask")
        ident_sb = singles.tile([P, P], F16, tag="ident")
        ones_sb = singles.tile([97, DH], F16, tag="ones")
        nc.vector.memset(ones_sb[:], 1.0)

        # ---- big SBUF-resident tensors ----
        qt_sb = big.tile([P, 4, L], F16, tag="qt")       # [p,t,l] = Qt[t*128+p, l]
        kt_sb = big.tile([P, 4, L], F16, tag="kt")
        vx_sb = big.tile([P, NKT, H, DH + 1], F16, tag="vx")  # V + mask column
        attnT = [
            big.tile([P, 4, QC], F16, tag=f"attnT{qc}", name=f"attnT{qc}")
            for qc in range(NQC)
        ]

        # ---- phase 1: projections (pools scoped so SBUF is reclaimed) ----
        proj_ctx = contextlib.ExitStack()
        stage = proj_ctx.enter_context(tc.tile_pool(name="stage", bufs=3))
        wpool = proj_ctx.enter_context(tc.tile_pool(name="wpool", bufs=3))

        def load_stage(src, eng):
            t = stage.tile([P, 2, L], F16, tag="stage")
            eng.dma_start(out=t[:], in_=src.rearrange("(t p) l -> p t l", p=P))
            return t

        def load_w(src, eng):
            t = wpool.tile([P, 2, DM], F16, tag="w")
            eng.dma_start(out=t[:], in_=src.rearrange("(t p) d -> p t d", p=P))
            return t

        # finest-gating loads first: the first Qt matmul needs only the t=0
        # halves of wq and q, so issue those as separate DMAs; v/wv and all
        # prefetch go after the Qt/Kt matmuls are emitted
        wq_sb = wpool.tile([P, 2, DM], F16, tag="w", name="wq_sb")
        q_sb = stage.tile([P, 2, L], F16, tag="stage", name="q_sb")
        wqr = wqT.rearrange("(t p) d -> p t d", p=P)
        qr = qT.rearrange("(t p) l -> p t l", p=P)
        for tt in range(2):
            nc.sync.dma_start(out=wq_sb[:, tt, :], in_=wqr[:, tt, :])
            nc.sync.dma_start(out=q_sb[:, tt, :], in_=qr[:, tt, :])
        k_sb, wk_sb = load_stage(kT, nc.sync), load_w(wkT, nc.sync)

        b_sb = [None] * H

        def load_bias(h):
            t = bpool.tile([P, NKT, L], F16, tag="bias")
            nc.sync.dma_start(out=t[:], in_=bT_r[:, h, :, :])
            b_sb[h] = t

        # Qt / Kt: out[m=dm-tile, n=l-chunk] = sum_din w?T[din, dm] * xT[din, l]
        # (bias/wo DMAs are emitted AFTER these matmuls so the 4MB+ of
        # prefetch doesn't compete with the loads that gate the first matmul)
        for x_sb, w_sb, dst in ((q_sb, wq_sb, qt_sb), (k_sb, wk_sb, kt_sb)):
            for mt in range(4):
                ps = spsum.tile([P, 2 * QC], F32, tag="sp", name="ps")
                for lc in range(NQC):
                    for kt2 in range(2):
                        nc.tensor.matmul(
                            ps[:, lc * QC : (lc + 1) * QC],
                            w_sb[:, kt2, mt * P : (mt + 1) * P],
                            x_sb[:, kt2, lc * QC : (lc + 1) * QC],
                            start=(kt2 == 0),
                            stop=(kt2 == 1),
                        )
                nc.vector.tensor_copy(out=dst[:, mt, :], in_=ps[:])

        v_sb, wv_sb = load_stage(vT, nc.sync), load_w(wvT, nc.sync)
        nc.sync.dma_start(out=mask_sb[:], in_=mask01[:])
        nc.sync.dma_start(out=ident_sb[:], in_=identI[:])
        wo_sb = singles.tile([P, 4, DM], F16, tag="wo")
        nc.sync.dma_start(out=wo_sb[:], in_=woT.rearrange("(t p) d -> p t d", p=P))
        load_bias(0)
        load_bias(1)

        # V: out[m=l-tile, n=dm] = sum_din vT[din, l] * wvT[din, dm]; mask rows
        for lt in range(NKT):
            ps = pvp.tile([P, DM], F32, tag="pv")
            for kt2 in range(2):
                nc.tensor.matmul(
                    ps[:],
                    v_sb[:, kt2, lt * P : (lt + 1) * P],
                    wv_sb[:, kt2, :],
                    start=(kt2 == 0),
                    stop=(kt2 == 1),
                )
            nc.vector.tensor_scalar_mul(
                out=vx_sb[:, lt, :, 0:DH],
                in0=ps.rearrange("p (h d) -> p h d", h=H),
                scalar1=mask_sb[:, lt : lt + 1],
            )
            # mask column (softmax denominator counts only unmasked keys)
            nc.vector.tensor_copy(
                out=vx_sb[:, lt, :, DH : DH + 1],
                in_=mask_sb[:, lt : lt + 1, None].to_broadcast((P, H, 1)),
            )

        proj_ctx.close()

        # ---- phase 2: attention, software-pipelined ----
        # Work queue of deferred closures (reciprocals, normalizations, WO).
        # One item pops at each flush point; emission order = engine order, so
        # deferred items land in the PE stream well after their DVE/DMA
        # producers have had time to run.
        pending = []

        def flush_one():
            if pending:
                item = pending.pop(0)
                if item is not None:
                    item()

        # per-qc staging: unnormalized PV (+denom row) for all 8 heads, and
        # denominator gather tiles at partitions {0,32,64} so one DVE
        # reciprocal serves 3 heads (128-lane parallelism needs the
        # denominators spread across partitions; {0,32,64} are the only legal
        # base partitions for the bcast matmul operands)
        HGRP = [(0, 1, 2), (3, 4, 5), (6, 7)]  # head groups per den tile
        NG = len(HGRP)
        pvsq = [
            big.tile([DH + 1, H, QC], F32, tag=f"pvq{qc}", name=f"pvq{qc}")
            for qc in range(NQC)
        ]
        dens = [
            big.tile([65, QC], F32, tag=f"den{i}", name=f"den{i}")
            for i in range(NG * NQC)
        ]
        recbs = [
            big.tile([65, QC], F16, tag=f"recb{i}", name=f"recb{i}")
            for i in range(NG * NQC)
        ]

        def emit_recip(i, last=False):
            def go():
                with nc.allow_low_precision(
                    reason="fp16 softmax recip matches kernel precision"
                ):
                    nc.vector.reciprocal(out=recbs[i][:], in_=dens[i][:])

            pending.append(go)
            # pacing bubbles: the first dependent bcast pops ~3 flush points
            # (~5-7us of PE work) after the reciprocal is emitted
            pending.append(None)
            pending.append(None)
            pending.append(None)

        def emit_norm(h, qc):
            ht, odd = h // 2, h % 2
            rp = 32 * (h % 3)  # partition of this head's denominator row
            recb = recbs[NG * qc + h // 3]

            def go():
                bps = bcp.tile([DH, QC], F32, tag="bps")
                nc.tensor.matmul(
                    bps[:],
                    ones_sb[rp : rp + 1, :],
                    recb[rp : rp + 1, :],
                    start=True,
                    stop=True,
                )
                if not odd:
                    nc.vector.tensor_mul(
                        out=attnT[qc][0:DH, ht, :],
                        in0=pvsq[qc][0:DH, h, :],
                        in1=bps[:],
                    )
                else:
                    tmp = small.tile([DH, QC], F16, tag="odd")
                    nc.vector.tensor_mul(
                        out=tmp[:], in0=pvsq[qc][0:DH, h, :], in1=bps[:]
                    )
                    nc.sync.dma_start(out=attnT[qc][DH:P, ht, :], in_=tmp[:])

            pending.append(go)

        def wo_chain(qc, ws, ltp, kts):
            for i in range(2):
                lt = 2 * ltp + i
                for kt4 in kts:
                    nc.tensor.matmul(
                        ws[:, i * QC : (i + 1) * QC],
                        attnT[qc][:, kt4, lt * P : (lt + 1) * P],
                        wo_sb[:, kt4, :],
                        start=(kt4 == 0),
                        stop=(kt4 == 3),
                        skip_group_check=True,
                    )

        def wo_evac(qc, ws, ltp):
            # alternate evac engine so the two 1us copies run in parallel
            ost = small.tile([P, 2 * QC], F32, tag="ost")
            if ltp == 0:
                nc.scalar.copy(out=ost[:], in_=ws[:])
            else:
                nc.vector.tensor_copy(out=ost[:], in_=ws[:])
            for i in range(2):
                lt = 2 * ltp + i
                nc.sync.dma_start(
                    out=out[qc * QC + lt * P : qc * QC + (lt + 1) * P, :],
                    in_=ost[:, i * QC : (i + 1) * QC],
                )

        def emit_wo(qc):
            def go():
                for ltp in range(2):  # pairs of 128-row output tiles
                    ws = spsum.tile([P, 2 * QC], F32, tag="sp", name="ws")
                    wo_chain(qc, ws, ltp, range(4))
                    wo_evac(qc, ws, ltp)

            pending.append(go)

        def emit_wo_split(qc):
            # final-qc WO: the first 3 dm-chunks only need heads 0-5, so they
            # run on the PE while the last head-group's reciprocal chain is
            # still in flight; the stop matmuls + evac follow the last norms
            ws_tiles = []

            def begin():
                for ltp in range(2):
                    ws = spsum.tile([P, 2 * QC], F32, tag="sp", name="ws")
                    wo_chain(qc, ws, ltp, range(3))
                    ws_tiles.append(ws)

            def end():
                for ltp in range(2):
                    wo_chain(qc, ws_tiles[ltp], ltp, [3])
                    wo_evac(qc, ws_tiles[ltp], ltp)

            return begin, end

        for h in range(H):
            hb = (h % 2) * DH  # partition base of head h inside its dm-tile
            ht = h // 2
            if h + 2 < H:
                load_bias(h + 2)
            bh = b_sb[h]
            for qc in range(NQC):
                qs = slice(qc * QC, (qc + 1) * QC)
                # -- scores: QK (start) + single identity bias matmul (stop) --
                ex = exps.tile([P, NKT, QC], F16, tag="ex")
                for ktp in range(NKT // 2):  # pairs of k-tiles share a psum
                    sp = spsum.tile([P, 2 * QC], F32, tag="sp")
                    for i in range(2):
                        kt = 2 * ktp + i
                        nc.tensor.matmul(
                            sp[:, i * QC : (i + 1) * QC],
                            kt_sb[hb : hb + DH, ht, kt * P : (kt + 1) * P],
                            qt_sb[hb : hb + DH, ht, qs],
                            start=True,
                            stop=False,
                        )
                    for i in range(2):
                        kt = 2 * ktp + i
                        nc.tensor.matmul(
                            sp[:, i * QC : (i + 1) * QC],
                            ident_sb[:],
                            bh[:, kt, qs],
                            start=False, stop=True, skip_group_check=True,
                        )
                    nc.scalar.activation(
                        out=ex[:, 2 * ktp : 2 * ktp + 2, :].rearrange(
                            "p a b -> p (a b)"
                        ),
                        in_=sp[:],
                        func=mybir.ActivationFunctionType.Exp,
                    )
                flush_one()
                # -- PV with appended mask column -> row 64 = denominator --
                pv = pvp.tile([DH + 1, QC], F32, tag="pv")
                for kt in range(NKT):
                    nc.tensor.matmul(
                        pv[:],
                        vx_sb[:, kt, h, :],
                        ex[:, kt, :],
                        start=(kt == 0),
                        stop=(kt == NKT - 1),
                    )
                # stage PV to SBUF (frees the PSUM bank early); gather the
                # denominator row into the group reciprocal tile at a
                # 32-stride partition via a tiny SBUF->SBUF DMA
                nc.vector.tensor_copy(out=pvsq[qc][:, h, :], in_=pv[:])
                nc.sync.dma_start(
                    out=dens[NG * qc + h // 3][32 * (h % 3) : 32 * (h % 3) + 1, :],
                    in_=pvsq[qc][DH : DH + 1, h, :],
                )
                if h in (2, 5, 7):
                    g = h // 3
                    final = h == H - 1 and qc == NQC - 1
                    emit_recip(NG * qc + g, last=final)
                    if final:
                        wo_begin, wo_end = emit_wo_split(qc)
                        pending.append(wo_begin)
                    for hh in HGRP[g]:
                        emit_norm(hh, qc)
                    if h == H - 1:
                        if final:
                            pending.append(wo_end)
                        else:
                            emit_wo(qc)
                flush_one()

        while pending:
            flush_one()


def build_nc():
    from concourse import bacc

    nc = bacc.Bacc("TRN2", target_bir_lowering=False, debug=False)
    with tile.TileContext(nc) as tc:
        _emit(tc)
    nc.compile()
    return nc


_NC = None


def _get_nc():
    global _NC
    if _NC is None:
        _NC = build_nc()
    return _NC


def make_in_maps(queries, keys, values, attention_mask, adjacency_matrix,
                 distance_matrix, W_q, W_k, W_v, W_o, lambda_a, lambda_d):
    f = np.float32
    h16 = np.float16
    c = np.ascontiguousarray
    wqT = c((W_q.astype(f) * f(0.125)).T).astype(h16)
    wkT = c(W_k.astype(f).T).astype(h16)
    wvT = c(W_v.astype(f).T).astype(h16)
    woT = c(W_o.astype(f).T).astype(h16)
    identI = np.eye(P, dtype=h16)
    la = np.asarray(lambda_a, dtype=f)
    ld = np.asarray(lambda_d, dtype=f)
    in_maps = []
    for b in range(B):
        aT = c(np.asarray(adjacency_matrix[b], dtype=f).T)
        dT = c(np.asarray(distance_matrix[b], dtype=f).T)
        bT = np.empty((H, L, L), dtype=h16)
        for h in range(H):
            bT[h] = (la[h] * aT + ld[h] * dT).astype(h16)
        in_maps.append({
            "qT": c(queries[b].astype(f).T).astype(h16),
            "kT": c(keys[b].astype(f).T).astype(h16),
            "vT": c(values[b].astype(f).T).astype(h16),
            "wqT": wqT, "wkT": wkT, "wvT": wvT, "woT": woT,
            "bT": bT.reshape(H * L, L),
            "mask01": c((attention_mask[b] > 0).astype(f).reshape(NKT, P).T),
            "identI": identI,
        })
    return in_maps


def kernel(queries, keys, values, attention_mask, adjacency_matrix,
           distance_matrix, W_q, W_k, W_v, W_o, lambda_a, lambda_d, **kw):
    nc = _get_nc()
    in_maps = make_in_maps(queries, keys, values, attention_mask,
                           adjacency_matrix, distance_matrix,
                           W_q, W_k, W_v, W_o, lambda_a, lambda_d)
    res = run_bass_kernel_spmd(nc, in_maps, list(range(B)), **kw)
    outs = np.stack([res.results[i]["out"] for i in range(B)]).astype(np.float32)
    return outs
